# revision 27
# baseline (speedup 1.0000x reference)
"""Causal GQA self-attention (B=2, T=2048, D=2048, 16 q-heads / 4 kv-heads,
head_dim=128, full-dim RoPE) on 8 Trainium2 NeuronCores.

Strategy: tensor-parallel over (kv-head, batch). Pair p = cores {2p, 2p+1}
owns q-heads {4p..4p+3} and kv-head p; core 2p handles batch 0, core 2p+1
batch 1. Each core therefore projects Q (4 heads), K and V (1 kv head) for
its own 2048 tokens only — no duplicated K/V work and no cross-core
communication. Each core emits a full-width [2048, C] partial of the output
projection (bf16) and the host sums the 4 partials per batch.

On-chip layout: x is staged transposed and chunk-major ([128, chunk, kt,
512]) so every xt DMA piece reads 4KB-contiguous rows; all input DMAs are
issued on the single Sync queue in consumption order (per-queue FIFO is the
only priority mechanism the DMA ring offers). Attention is computed
"k-major" (scores transposed, [k_pos, q_pos]) so the P@V contraction needs
no transpose. V is projected with 512-row matmuls into vT and flipped to
token-major with two XBAR dma transposes (free for the PE). Softmax runs
without max-subtraction (scores are ~N(0,1); exp never overflows).

The attention inner loop is engine-balanced: exp on the ACT engine matches
PE throughput element-for-element (1 elem/cyc @1.2GHz vs 2 matmuls
@2.4GHz), so ACT fixed costs are amortized by exping PAIRS of k-tiles from
a single two-bank [128,1024] PSUM scores tile. q/y head tensors are
per-head tiles so dependency tracking is exact, and every out-projection
window streams the head contraction kd-major across 2 PSUM groups — the
last head's reciprocal->normalize chain (DVE) overlaps earlier heads'
partial products. PSUM drains are spread over ACT/DVE/GpSimd per window so
no single vector engine outruns its exp/denominator budget.
"""

import math
import os
import sys

for _p in ("/opt/trn_rl_repo", "/root/.axon_site/_ro/trn_rl_repo"):
    if os.path.isdir(_p) and _p not in sys.path:
        sys.path.insert(0, _p)

import ml_dtypes
import numpy as np

BF16 = ml_dtypes.bfloat16

B = 2
T = 2048         # tokens per core (= one full batch sequence)
C = 2048
D = 128          # head dim
NQH = 4          # q heads per core
KT = C // 128    # 16 contraction tiles
NCH = 512        # matmul moving-dim chunk
QCH = T // NCH   # 4 q chunks
KB = T // 128    # 16 k tiles
N_CORES = 8
SCALE = 1.0 / math.sqrt(D)

_COMPILED = {}


def _rope_tables():
    dim = np.arange(D // 2, dtype=np.float64)
    freq = 10000.0 ** (dim / (D / 2))
    freq = np.concatenate([freq, freq])              # [128]
    pos = np.arange(T, dtype=np.float64)
    ang = pos[None, :] / freq[:, None]               # [128, T] channel-major
    return np.cos(ang), np.sin(ang)


def _build_nc(debug=False):
    import concourse.bass as bass  # noqa: F401
    import concourse.mybir as mybir
    import concourse.tile as tile
    from concourse import bacc
    from concourse.bass import ts

    f32 = mybir.dt.float32
    bf16 = mybir.dt.bfloat16
    AF = mybir.ActivationFunctionType
    OP = mybir.AluOpType

    nc = bacc.Bacc("TRN2", target_bir_lowering=False, debug=False,
                   num_devices=N_CORES)

    xt_e = nc.dram_tensor("xt", [128, QCH * KT * NCH], bf16, kind="ExternalInput")
    wq_e = nc.dram_tensor("wq", [128, NQH * KT * D], bf16, kind="ExternalInput")
    wk_e = nc.dram_tensor("wk", [128, KT * D], bf16, kind="ExternalInput")
    wv_e = nc.dram_tensor("wv", [128, KT * D], bf16, kind="ExternalInput")
    wp_e = nc.dram_tensor("wp", [128, NQH * C], bf16, kind="ExternalInput")
    cos_e = nc.dram_tensor("cos", [D, T], bf16, kind="ExternalInput")
    sin_e = nc.dram_tensor("sin", [D, T], bf16, kind="ExternalInput")
    tri_e = nc.dram_tensor("tri", [D, D], bf16, kind="ExternalInput")
    out_e = nc.dram_tensor("out", [T, C], bf16, kind="ExternalOutput")

    from contextlib import ExitStack

    with tile.TileContext(nc) as tc, ExitStack() as ctx:
        const = ctx.enter_context(tc.tile_pool(name="const", bufs=1))
        qkvp = ctx.enter_context(tc.tile_pool(name="qkv", bufs=1))
        psum = ctx.enter_context(tc.tile_pool(name="ps", bufs=3, space="PSUM"))
        xtp = ctx.enter_context(tc.tile_pool(name="xt", bufs=1))
        w1p = ctx.enter_context(tc.tile_pool(name="w1", bufs=1))
        rtp = ctx.enter_context(tc.tile_pool(name="rt", bufs=3))
        exp_p = ctx.enter_context(tc.tile_pool(name="exp", bufs=6))
        denp = ctx.enter_context(tc.tile_pool(name="den", bufs=2))
        recp = ctx.enter_context(tc.tile_pool(name="rec", bufs=2))
        outp = ctx.enter_context(tc.tile_pool(name="outs", bufs=4))

        # All input DMAs ride the Sync queue in exact consumption order.
        wk_sb = w1p.tile([128, KT, D], bf16, tag="wk")
        nc.sync.dma_start(wk_sb[:], wk_e.ap().rearrange("p (ko n) -> p ko n", ko=KT))
        xt_sb = xtp.tile([128, QCH, KT, NCH], bf16, tag="xt")
        xt_r = xt_e.ap().rearrange("p (c ko t) -> p c ko t", c=QCH, ko=KT)
        nc.sync.dma_start(xt_sb[:, 0, 0:8, :], xt_r[:, 0, 0:8, :])
        wv_sb = w1p.tile([128, KT, D], bf16, tag="wv")
        nc.sync.dma_start(wv_sb[:], wv_e.ap().rearrange("p (ko n) -> p ko n", ko=KT))
        nc.sync.dma_start(xt_sb[:, 0, 8:16, :], xt_r[:, 0, 8:16, :])
        wq_sb = w1p.tile([128, NQH, KT, D], bf16, tag="wq")
        wq_r = wq_e.ap().rearrange("p (h ko n) -> p h ko n", h=NQH, ko=KT)
        for h in range(NQH):
            nc.sync.dma_start(wq_sb[:, h], wq_r[:, h])
        cos_sb = const.tile([D, T], bf16, tag="cos")
        nc.sync.dma_start(cos_sb[:], cos_e.ap())
        sin_sb = const.tile([D, T], bf16, tag="sin")
        nc.sync.dma_start(sin_sb[:], sin_e.ap())
        tri_sb = const.tile([D, D], bf16, tag="tri")
        nc.sync.dma_start(tri_sb[:], tri_e.ap())
        ones_sb = const.tile([128, 128], bf16, tag="ones")
        nc.vector.memset(ones_sb[:], 1.0)
        for tc_ in range(1, QCH):
            for g in range(2):
                nc.sync.dma_start(xt_sb[:, tc_, 8 * g:8 * g + 8, :],
                                  xt_r[:, tc_, 8 * g:8 * g + 8, :])
        wp_sb = const.tile([128, NQH, C], bf16, tag="wp")
        nc.sync.dma_start(wp_sb[:], wp_e.ap().rearrange("p (ko n) -> p ko n", ko=NQH))

        # persistent tensors; q/y are per-head tiles so readers depend only
        # on the head they actually consume.
        qTs = [qkvp.tile([D, T], bf16, tag=f"qT{h}", name=f"qT{h}")
               for h in range(NQH)]
        kT = qkvp.tile([D, T], bf16, tag="kT")
        vT = qkvp.tile([D, T], bf16, tag="vT")
        vv = qkvp.tile([128, KB, D], bf16, tag="vv")
        yTs = [qkvp.tile([D, T], bf16, tag=f"yT{h}", name=f"yT{h}")
               for h in range(NQH)]

        def rope_out(dst, src_ps, cos_ap, sin_ap):
            """dst(bf16 sbuf) = src * cos + rotate_half(src) * sin."""
            sb = rtp.tile([128, NCH], bf16, tag="sb")
            nc.scalar.copy(sb[:], src_ps)
            rt = rtp.tile([128, NCH], bf16, tag="rt")
            nc.vector.tensor_scalar(out=rt[0:64, :], in0=sb[64:128, :],
                                    scalar1=-1.0, scalar2=None, op0=OP.mult)
            nc.vector.tensor_copy(out=rt[64:128, :], in_=sb[0:64, :])
            m1 = rtp.tile([128, NCH], bf16, tag="m1")
            nc.vector.tensor_tensor(out=m1[:], in0=sb[:], in1=cos_ap, op=OP.mult)
            m2 = rtp.tile([128, NCH], bf16, tag="m2")
            nc.vector.tensor_tensor(out=m2[:], in0=rt[:], in1=sin_ap, op=OP.mult)
            nc.vector.tensor_tensor(out=dst, in0=m1[:], in1=m2[:], op=OP.add)

        def emit_proj(qc, n_act, final=False):
            # Out-projection window. PSUM accumulation tiles alternate
            # between the "mm" and "y" tags — the y banks are idle exactly
            # when proj runs, giving a 4-bank rotation so drains overlap the
            # next group's matmuls. n_act: how many of the 16 drains run on
            # the ACT engine (rest on DVE; GpSimd cannot read PSUM), tuned
            # per window against the exp/denominator budget. The final
            # window additionally streams the head contraction kd-major
            # across 2 concurrent groups so the last heads' normalize
            # chains stay off the critical path, and DMAs each 128x512
            # piece the moment it drains.
            groups = [(qt, fc) for qt in range(4 * qc, 4 * qc + 4)
                      for fc in range(C // NCH)]
            osbs = {}
            for qt in range(4 * qc, 4 * qc + 4):
                osb = outp.tile([128, C], bf16, tag="osb")
                osbs[qt] = osb

            act_set = {(i * 16) // n_act for i in range(n_act)} if n_act else set()

            def drain(gi, ops, qt, fc):
                osb = osbs[qt]
                if gi in act_set:
                    nc.scalar.copy(osb[:, ts(fc, NCH)], ops[:])
                else:
                    nc.vector.tensor_copy(osb[:, ts(fc, NCH)], ops[:])
                if final:
                    deng = nc.sync if gi % 2 == 0 else nc.gpsimd
                    deng.dma_start(
                        out_e.ap()[qt * 128:(qt + 1) * 128, ts(fc, NCH)],
                        osb[:, ts(fc, NCH)])
                elif fc == 3:
                    nc.sync.dma_start(
                        out_e.ap()[qt * 128:(qt + 1) * 128, :], osb[:])

            if not final:
                for gi, (qt, fc) in enumerate(groups):
                    ops = psum.tile([128, NCH], f32, tag="mm",
                                    name=f"pj{qc}_{gi}")
                    for kd in range(NQH):
                        nc.tensor.matmul(
                            ops[:],
                            yTs[kd][:, qt * 128:(qt + 1) * 128],
                            wp_sb[:, kd, ts(fc, NCH)],
                            start=(kd == 0), stop=(kd == NQH - 1))
                    drain(gi, ops, qt, fc)
            else:
                for g0 in range(0, len(groups), 3):
                    batch = groups[g0:g0 + 3]
                    opss = [psum.tile([128, NCH], f32, tag="mm",
                                      name=f"pj{qc}_{g0}_{i}")
                            for i in range(len(batch))]
                    for kd in range(NQH):
                        for ops, (qt, fc) in zip(opss, batch):
                            nc.tensor.matmul(
                                ops[:],
                                yTs[kd][:, qt * 128:(qt + 1) * 128],
                                wp_sb[:, kd, ts(fc, NCH)],
                                start=(kd == 0), stop=(kd == NQH - 1))
                    for i, (ops, (qt, fc)) in enumerate(zip(opss, batch)):
                        drain(g0 + i, ops, qt, fc)

        # PE warm-up: the first ~13us are DMA-bound (nothing for the PE to
        # chew on), which lets the tensor engine's p-state decay; a chain of
        # dummy matmuls on the ones tile keeps it clocked at full speed so
        # the first real chains don't pay the ramp. The chain occupies one
        # "mm" PSUM buffer that is never read; it finishes inside the DMA
        # window so it delays nothing.
        wsrc = const.tile([128, NCH], bf16, tag="wsrc")
        nc.vector.memset(wsrc[:], 0.5)
        warm = psum.tile([128, NCH], f32, tag="mm")
        for w in range(44):
            nc.tensor.matmul(warm[:], ones_sb[:], wsrc[:],
                             start=(w == 0), stop=(w == 43))

        # ---- phase 1: QKV projection + RoPE ----
        for tc_ in range(QCH):
            ps = psum.tile([128, NCH], f32, tag="mm")
            for kt in range(KT):
                nc.tensor.matmul(ps[:], wk_sb[:, kt, :],
                                 xt_sb[:, tc_, kt, :],
                                 start=(kt == 0), stop=(kt == KT - 1))
            rope_out(kT[:, ts(tc_, NCH)],
                     ps, cos_sb[:, ts(tc_, NCH)], sin_sb[:, ts(tc_, NCH)])
            ps = psum.tile([128, NCH], f32, tag="mm")
            for kt in range(KT):
                nc.tensor.matmul(ps[:], wv_sb[:, kt, :],
                                 xt_sb[:, tc_, kt, :],
                                 start=(kt == 0), stop=(kt == KT - 1))
            nc.scalar.copy(vT[:, ts(tc_, NCH)], ps[:])
            if tc_ % 2 == 1:
                half = tc_ // 2
                nc.scalar.dma_start_transpose(
                    vv[:, 8 * half:8 * half + 8, :],
                    vT[:, half * 1024:(half + 1) * 1024])
            for h in range(NQH):
                ps = psum.tile([128, NCH], f32, tag="mm")
                for kt in range(KT):
                    nc.tensor.matmul(ps[:],
                                     wq_sb[:, h, kt, :],
                                     xt_sb[:, tc_, kt, :],
                                     start=(kt == 0), stop=(kt == KT - 1))
                rope_out(qTs[h][:, ts(tc_, NCH)],
                         ps, cos_sb[:, ts(tc_, NCH)], sin_sb[:, ts(tc_, NCH)])

        # ---- phase 2+3: attention + out-projection ----
        # proj emission is delayed one chunk so the PE stream always has the
        # next attention chunk ahead of each proj. Full k-tiles are
        # processed in PAIRS sharing one two-bank PSUM scores tile and one
        # exp; the diagonal tiles stay singles (column-skipped).
        DRAIN_MIX = {1: 8, 2: 8, 3: 4}
        pending = None
        for qc in range(QCH):
            for h in range(NQH):
                n_kt = 4 * qc + 4
                yps = psum.tile([128, NCH], f32, tag="y", bufs=2)
                den = denp.tile([128, NCH], bf16, tag="den")
                for kti in range(n_kt):
                    dq = kti - 4 * qc
                    c0 = dq * 128 if dq > 0 else 0  # masked cols skipped
                    sc = psum.tile([128, NCH], f32, tag="sc")
                    ex = exp_p.tile([128, NCH], bf16, tag="ex")
                    nc.tensor.matmul(sc[:, c0:],
                                     kT[:, kti * 128:(kti + 1) * 128],
                                     qTs[h][:, qc * NCH + c0:(qc + 1) * NCH],
                                     start=True, stop=True)
                    nc.scalar.activation(ex[:, c0:], sc[:, c0:], AF.Exp)
                    if dq >= 0:
                        # causal mask on the (otherwise idle) Pool engine;
                        # the last diagonal runs on DVE so the denominator
                        # reduce isn't gated on a Pool round-trip
                        eng_tri = nc.vector if dq == 3 else nc.gpsimd
                        eng_tri.tensor_mul(ex[:, ts(dq, 128)],
                                           ex[:, ts(dq, 128)], tri_sb[:])
                    # denominator partial-sum on DVE (bf16, 2x rate)
                    if kti == 0:
                        nc.vector.tensor_copy(den[:], ex[:])
                    else:
                        nc.vector.tensor_tensor(out=den[:, c0:],
                                                in0=den[:, c0:],
                                                in1=ex[:, c0:], op=OP.add)
                    nc.tensor.matmul(yps[:, c0:], vv[:, kti, :],
                                     ex[:, c0:],
                                     start=(kti == 0), stop=(kti == n_kt - 1))
                # reduce denominator across partitions (one matmul), then
                # normalize yT straight out of PSUM on the DVE.
                dps = psum.tile([128, NCH], f32, tag="sc")
                nc.tensor.matmul(dps[:], ones_sb[:], den[:],
                                 start=True, stop=True)
                rec = recp.tile([128, NCH], f32, tag="rec")
                nc.vector.reciprocal_approx_fast(rec[:], dps[:])
                nc.vector.tensor_mul(
                    yTs[h][:, ts(qc, NCH)], yps[:], rec[:])
            if pending is not None:
                emit_proj(pending, DRAIN_MIX[qc])
            pending = qc
        emit_proj(pending, 8, final=True)

    nc.compile()
    return nc


def _get_nc():
    if "nc" not in _COMPILED:
        _COMPILED["nc"] = _build_nc()
    return _COMPILED["nc"]


def _stage_inputs(x, Wq, Wkv, Wproj):
    cos, sin = _rope_tables()
    cos = cos.astype(BF16)
    sin = sin.astype(BF16)
    kk, qq = np.meshgrid(np.arange(D), np.arange(D), indexing="ij")
    tri = (kk <= qq).astype(BF16)                               # [k, q]

    def pmaj(w):
        k, n = w.shape
        return np.ascontiguousarray(
            w.reshape(k // 128, 128, n).transpose(1, 0, 2)
            .reshape(128, (k // 128) * n))

    def stage_xt(xb):
        # [T, C] -> channel-major, chunk-major [128, QCH*KT*NCH]: element
        # (p, c, ko, t) = x[c*NCH + t, ko*128 + p]
        xt = xb.T.astype(BF16)                       # [C, T]
        xt = xt.reshape(KT, 128, QCH, NCH)           # [ko, p, c, t]
        return np.ascontiguousarray(
            xt.transpose(1, 2, 0, 3).reshape(128, QCH * KT * NCH))

    xts = [stage_xt(x[b]) for b in range(B)]
    in_maps = []
    for c in range(N_CORES):
        p, b = c // 2, c % 2
        wq = np.concatenate(
            [pmaj((Wq[(4 * p + h) * D:(4 * p + h + 1) * D, :] * SCALE)
                  .T.astype(BF16)) for h in range(NQH)], axis=1)
        wk = pmaj(Wkv[p * D:(p + 1) * D, :].T.astype(BF16))
        wv = pmaj(Wkv[4 * D + p * D: 4 * D + (p + 1) * D, :].T.astype(BF16))
        wp = pmaj(Wproj[:, 4 * p * D:(4 * p + 4) * D].T.astype(BF16))
        in_maps.append({
            "xt": xts[b], "wq": wq, "wk": wk, "wv": wv, "wp": wp,
            "cos": cos, "sin": sin, "tri": tri,
        })
    return in_maps


def run(x, Wq, Wkv, Wproj, trace=False):
    from concourse.bass_utils import run_bass_kernel_spmd

    nc = _get_nc()
    in_maps = _stage_inputs(x, Wq, Wkv, Wproj)
    res = run_bass_kernel_spmd(nc, in_maps, core_ids=list(range(N_CORES)),
                               trace=trace)
    out = np.zeros((B, T, C), np.float32)
    for c in range(N_CORES):
        out[c % 2] += res.results[c]["out"].astype(np.float32)
    return (out, res) if trace else (out, None)


def kernel(x, Wq, Wkv, Wproj):
    out, _ = run(np.asarray(x, np.float32), np.asarray(Wq, np.float32),
                 np.asarray(Wkv, np.float32), np.asarray(Wproj, np.float32))
    return out
